# revision 42
# baseline (speedup 1.0000x reference)
"""Multi-head attention kernel for 8 TRN2 NeuronCores (raw Bass, no Tile).

Problem: x[2,4096,256] -> qkv proj -> 8-head attention (Dh=32) -> out proj.
Sharding: 16 (batch, head) pairs over 8 cores: core i handles batch i//4 and
heads {2*(i%4), 2*(i%4)+1}. Each core computes its 2 heads' attention plus the
partial output projection; host sums the 4 partial projections per batch.

All matmul operands are bf16 (1 PE cycle/row vs 4 for fp32); PSUM accumulation
stays fp32. The softmax exp runs on ScalarE from PSUM (fp32 in, bf16 out) and
is the steady-state bottleneck (~1.1µs per [128,1024] tile).

Semaphore waits are EMBEDDED into the consuming instructions (one wait per
instruction; walrus moves matmul waits onto the paired LDWEIGHTS) instead of
standalone EVENT_SEMAPHORE instructions: gaps in the PE instruction stream
make the HAM activity monitor hold the PE clock at K=4/8 (1.2 GHz instead of
2.4), which doubled tensor time in the gap-heavy variant. pt (exp output) and
den_r are 4-deep so their anti-dependency waits are implied by the in-order
tensor queue and can be dropped entirely.

Per-core dataflow:
  xT [256,4096] bf16 (features on partitions); q/k biases added by DVE
  tensor_scalar_add during the PSUM->SBUF cast (no bias matmuls); softmax
  scale folded into Wq host-side.
  qT/kT [64,4096] bf16 (dh on partitions, head h at rows 32h:32h+32).
  v packed [128, 32*66] bf16: per k-block [v_h0(32)|1|v_h1(32)|1]: the ones
  column makes the AV matmul (M=33) also produce the softmax denominator row.
  Scores transposed: S^T[128 kpos, 2*512 q] = kT_blk.T @ qT_chunk (PSUM,
  double-buffered), exp on ScalarE -> pt bf16, AV: po[33,512] += v_blk.T @ P^T
  across 32 k-blocks, denominator reciprocal (reciprocal_approx_fast, ~18
  correct bits) read straight from PSUM, broadcast to 32 partitions via K=1
  ones matmul, normalize on VectorE (reads po PSUM directly), project, DMA.

Pipeline: PE runs scores(g+1) before AV(g) so ScalarE's exp(g) overlaps;
ping-pong pairs for score PSUM, po PSUM, and all epilogue buffers.
"""

import sys

sys.path.insert(0, "/opt/trn_rl_repo")

import numpy as np
import ml_dtypes

BF16 = ml_dtypes.bfloat16

B, N, C, H, Dh = 2, 4096, 256, 8, 32
HPC = 2  # heads per core
NCORES = 8
SCALE = C ** -0.5
QC = 512  # q columns per chunk
NQC = N // QC  # 8
KB = 128  # k rows per block
NKB = N // KB  # 32
VW = 2 * Dh + 2  # packed v width per k-block
NI = NQC * HPC  # 16 (c, h) iterations
NT = NKB // 2  # 16 double-kb tiles per iteration
NG = NI * NT  # 256 score/exp/AV tiles

_CACHE = {}


def _build_nc():
    import concourse.bass as bass
    import concourse.mybir as mybir

    f32 = mybir.dt.float32
    bf16 = mybir.dt.bfloat16
    EXP = mybir.ActivationFunctionType.Exp

    nc = bass.Bass("TRN2", target_bir_lowering=False, debug=False, num_devices=NCORES)

    ins = {}
    for nm, shp, dt in (
        ("x0", [128, N], bf16), ("x1", [128, N], bf16),
        ("wq0", [128, 2 * Dh], bf16), ("wq1", [128, 2 * Dh], bf16),
        ("wqb", [2 * Dh, 1], f32),
        ("wk0", [128, 2 * Dh], bf16), ("wk1", [128, 2 * Dh], bf16),
        ("wkb", [2 * Dh, 1], f32),
        ("wv0", [128, 2 * Dh], bf16), ("wv1", [128, 2 * Dh], bf16),
        ("wvb", [1, 2 * Dh], bf16),
        ("wp0", [Dh, C], bf16), ("wp1", [Dh, C], bf16),
    ):
        ins[nm] = nc.dram_tensor(nm, shp, dt, kind="ExternalInput")
    out = nc.dram_tensor("out", [C, N], f32, kind="ExternalOutput")

    from contextlib import ExitStack
    with ExitStack() as ctx:
        E = ctx.enter_context
        E(nc.allow_low_precision(reason="bf16 matmul operands; fp32 PSUM accum"))
        xt0 = E(nc.sbuf_tensor("xt0", [128, N], bf16))
        xt1 = E(nc.sbuf_tensor("xt1", [128, N], bf16))
        twq0 = E(nc.sbuf_tensor("twq0", [128, 2 * Dh], bf16))
        twq1 = E(nc.sbuf_tensor("twq1", [128, 2 * Dh], bf16))
        tbq = E(nc.sbuf_tensor("tbq", [2 * Dh, 1], f32))
        twk0 = E(nc.sbuf_tensor("twk0", [128, 2 * Dh], bf16))
        twk1 = E(nc.sbuf_tensor("twk1", [128, 2 * Dh], bf16))
        tbk = E(nc.sbuf_tensor("tbk", [2 * Dh, 1], f32))
        twv0 = E(nc.sbuf_tensor("twv0", [128, 2 * Dh], bf16))
        twv1 = E(nc.sbuf_tensor("twv1", [128, 2 * Dh], bf16))
        twvb = E(nc.sbuf_tensor("twvb", [1, 2 * Dh], bf16))
        # proj weights / oT / bcast operands are also K=128 zero-padded (see
        # qTh comment: row-limited weights de-warm the HAM clock gate)
        twp0 = E(nc.sbuf_tensor("twp0", [128, C], bf16))
        twp1 = E(nc.sbuf_tensor("twp1", [128, C], bf16))
        ones_sm = E(nc.sbuf_tensor("ones_sm", [1, QC], bf16))
        ones_bc = E(nc.sbuf_tensor("ones_bc", [128, Dh], f32))
        # scores contraction is padded to K=128 with zero rows: matmuls whose
        # weights use only a 32-row strip of the PE array don't register as
        # "busy" to the HAM clock gate, which then holds the PE at K=4/8
        # (1.2 GHz). kT holds both heads' k at rows 0..63, zeros above; qTh[h]
        # holds head h's q at rows 32h..32h+32, zeros elsewhere, so the cross
        # terms vanish. Costs nothing: matmul time scales with moving columns.
        qTh0 = E(nc.sbuf_tensor("qTh0", [128, N], bf16))
        qTh1 = E(nc.sbuf_tensor("qTh1", [128, N], bf16))
        kT = E(nc.sbuf_tensor("kT", [128, N], bf16))
        qTh_ = (qTh0, qTh1)
        v_all = E(nc.sbuf_tensor("v_all", [128, NKB * VW], bf16))
        pt_ = tuple(
            E(nc.sbuf_tensor(f"pt{j}", [128, 2 * QC], bf16)) for j in range(4)
        )
        oT0 = E(nc.sbuf_tensor("oT0", [128, N], bf16))
        oT1 = E(nc.sbuf_tensor("oT1", [128, N], bf16))
        den_r_ = tuple(
            E(nc.sbuf_tensor(f"den_r{j}", [128, QC], f32)) for j in range(4)
        )
        den_sb0 = E(nc.sbuf_tensor("den_sb0", [Dh, QC], f32))
        den_sb1 = E(nc.sbuf_tensor("den_sb1", [Dh, QC], f32))
        osb0 = E(nc.sbuf_tensor("osb0", [128, QC], f32))
        osb1 = E(nc.sbuf_tensor("osb1", [128, QC], f32))
        s0 = E(nc.psum_tensor("s0", [128, 2 * QC], f32))
        s1 = E(nc.psum_tensor("s1", [128, 2 * QC], f32))
        po0 = E(nc.psum_tensor("po0", [Dh + 1, QC], f32))
        po1 = E(nc.psum_tensor("po1", [Dh + 1, QC], f32))
        pq0 = E(nc.psum_tensor("pq0", [128, QC], f32))
        pq1 = E(nc.psum_tensor("pq1", [128, QC], f32))
        dIN = E(nc.semaphore("dIN"))
        mset = E(nc.semaphore("mset"))
        sPQ = E(nc.semaphore("sPQ"))
        sCP = E(nc.semaphore("sCP"))
        sSC = E(nc.semaphore("sSC"))
        sEX = E(nc.semaphore("sEX"))
        sAV = E(nc.semaphore("sAV"))
        sRC = E(nc.semaphore("sRC"))
        sBC = E(nc.semaphore("sBC"))
        sDC = E(nc.semaphore("sDC"))
        sMU = E(nc.semaphore("sMU"))
        sPJ = E(nc.semaphore("sPJ"))
        sOS = E(nc.semaphore("sOS"))
        sOD0 = E(nc.semaphore("sOD0"))
        sOD1 = E(nc.semaphore("sOD1"))
        block = E(nc.Block())

        s_ = (s0, s1)
        po_ = (po0, po1)
        pq_ = (pq0, pq1)
        den_sb_ = (den_sb0, den_sb1)
        osb_ = (osb0, osb1)
        oT_ = (oT0, oT1)

        sb_of = {
            "x0": xt0, "x1": xt1, "wq0": twq0, "wq1": twq1, "wqb": tbq,
            "wk0": twk0, "wk1": twk1, "wkb": tbk, "wv0": twv0, "wv1": twv1,
            "wvb": twvb, "wp0": twp0, "wp1": twp1,
        }

        NMSET = 15

        @block.sync
        def _(sync):
            for nm, t in sb_of.items():
                if nm in ("wp0", "wp1"):
                    continue
                sync.dma_start(out=t[:, :], in_=ins[nm].ap()).then_inc(dIN, 16)
            # wp DMAs write rows 0..31 of the zero-padded twp buffers: wait
            # for the gpsimd memsets to avoid racing the zero fill
            sync.wait_ge(mset, NMSET)
            for nm in ("wp0", "wp1"):
                sync.dma_start(
                    out=sb_of[nm][0:Dh, :], in_=ins[nm].ap()
                ).then_inc(dIN, 16)
            for k in range(2 * NQC):  # out DMAs: c-major, mc-minor
                c, mc = k // 2, k % 2
                sync.wait_ge(sOS, k + 1)
                sync.dma_start(
                    out=out.ap()[mc * 128 : (mc + 1) * 128, c * QC : (c + 1) * QC],
                    in_=osb_[k % 2][:, :],
                ).then_inc((sOD0, sOD1)[k % 2], 16)

        @block.gpsimd
        def _(gpsimd):
            gpsimd.memset(ones_sm[:, :], 1.0).then_inc(mset, 1)
            gpsimd.memset(ones_bc[:, :], 0.0).then_inc(mset, 1)
            gpsimd.memset(ones_bc[0:1, :], 1.0).then_inc(mset, 1)
            gpsimd.memset(v_all[:, :], 1.0).then_inc(mset, 1)
            gpsimd.memset(qTh0[:, :], 0.0).then_inc(mset, 1)
            gpsimd.memset(qTh1[:, :], 0.0).then_inc(mset, 1)
            gpsimd.memset(kT[:, :], 0.0).then_inc(mset, 1)
            gpsimd.memset(oT0[:, :], 0.0).then_inc(mset, 1)
            gpsimd.memset(oT1[:, :], 0.0).then_inc(mset, 1)
            gpsimd.memset(twp0[:, :], 0.0).then_inc(mset, 1)
            gpsimd.memset(twp1[:, :], 0.0).then_inc(mset, 1)
            for j in range(4):
                gpsimd.memset(den_r_[j][:, :], 0.0).then_inc(mset, 1)

        # cumulative sCP count after production group g (q groups make 2
        # copies — one per head's padded buffer — k groups 1)
        cum = []
        tot = 0
        for g in range(2 * NQC):
            tot += 2 if g % 2 == 0 else 1
            cum.append(tot)
        CUM_QK = cum[-1]  # 24

        @block.tensor
        def _(tensor):
            tensor.wait_ge(dIN, 13 * 16)
            tensor.wait_ge(mset, NMSET)
            # ---- q/k projections: 16 groups (c-major; q then k) ----
            for g in range(2 * NQC):
                c, which = g // 2, g % 2
                sl = slice(c * QC, (c + 1) * QC)
                w0, w1 = ((twq0, twq1), (twk0, twk1))[which]
                p = pq_[g % 2][0 : 2 * Dh, :]
                mm = tensor.matmul(p, w0[:, :], xt0[:, sl], start=True, stop=False)
                if g >= 2:
                    mm._wait_ge(sCP, cum[g - 2])  # pq[g%2] free after copies g-2
                tensor.matmul(
                    p, w1[:, :], xt1[:, sl], start=False, stop=True
                ).then_inc(sPQ, 1)
            # ---- v projection: 32 blocks ----
            for pb in range(NKB):
                psl = slice(pb * KB, (pb + 1) * KB)
                p = pq_[pb % 2]
                pv = p[:, 0 : 2 * Dh]
                tensor.matmul(
                    pv, xt0[:, psl], twv0[:, :], start=True, stop=False
                )._wait_ge(sCP, CUM_QK + 2 * pb - 2 if pb >= 2 else CUM_QK)
                tensor.matmul(pv, xt1[:, psl], twv1[:, :], start=False, stop=False)
                tensor.matmul(
                    pv, ones_sm[0:1, 0:KB], twvb[:, :], start=False, stop=True
                ).then_inc(sPQ, 1)
            tensor.wait_ge(sCP, CUM_QK + 2 * NKB)  # all production copies done
            # ---- attention ----
            def scores(g):
                # no sEX wait: av(g-2), issued just before us in program
                # order, already waited on sEX >= g-1 (s[g%2] free)
                i, t = g // NT, g % NT
                c, h = i // 2, i % 2
                qsl = slice(c * QC, (c + 1) * QC)
                for j in (0, 1):
                    kb = 2 * t + j
                    ksl = slice(kb * KB, (kb + 1) * KB)
                    mm = tensor.matmul(
                        s_[g % 2][:, j * QC : (j + 1) * QC],
                        kT[:, ksl],
                        qTh_[h][:, qsl],
                        start=True,
                        stop=True,
                    )
                    if j == 1:
                        mm.then_inc(sSC, 1)

            def av(g):
                i, t = g // NT, g % NT
                h = i % 2
                if t == 0 and i >= 2:
                    tensor.wait_ge(sMU, i - 1)  # po[i%2] free after mul i-2
                for j in (0, 1):
                    kb = 2 * t + j
                    o = kb * VW + (Dh + 1) * h
                    mm = tensor.matmul(
                        po_[i % 2][:, :],
                        v_all[:, o : o + Dh + 1],
                        pt_[g % 4][:, j * QC : (j + 1) * QC],
                        start=(kb == 0),
                        stop=(kb == NKB - 1),
                        skip_group_check=True,
                    )
                    if j == 0:
                        mm._wait_ge(sEX, g + 1)  # exp g done
                    if j == 1:
                        mm.then_inc(sAV, 1)

            def bcast(i):
                if i >= 1:
                    tensor.wait_ge(sDC, i)  # pq1 free after den copy i-1
                tensor.matmul(
                    pq1[0:Dh, :],
                    ones_bc[:, :],
                    den_r_[i % 4][:, :],
                    start=True,
                    stop=True,
                )._wait_ge(sRC, i + 1).then_inc(sBC, 1)

            def proj(c):
                qsl = slice(c * QC, (c + 1) * QC)
                tensor.wait_ge(sMU, 2 * c + 2)  # oT ready through chunk c
                for mc in range(2):
                    k = 2 * c + mc
                    msl = slice(mc * 128, (mc + 1) * 128)
                    mm = tensor.matmul(
                        pq0[:, :], twp0[:, msl], oT0[:, qsl], start=True, stop=False
                    )
                    if k >= 1:
                        mm._wait_ge(sOS, k)  # pq0 free after osb copy k-1
                    tensor.matmul(
                        pq0[:, :], twp1[:, msl], oT1[:, qsl], start=False, stop=True
                    ).then_inc(sPJ, 1)

            for i in range(NI):
                for t in range(NT):
                    g = i * NT + t
                    scores(g)
                    if t >= 1:
                        av(g - 1)
                    if t == 4 and i >= 1:
                        # t==4: late enough that the ~3.3us DVE reciprocal of
                        # iter i-1 has finished (no PE stall on sRC), early
                        # enough that mul(i-1) lands this iteration, one full
                        # iteration before av(i+1, t=0) waits on sMU
                        bcast(i - 1)
                av(i * NT + NT - 1)
                if i >= 3 and i % 2 == 1 and i < NI - 1:
                    proj((i - 3) // 2)
            bcast(NI - 1)
            proj(NQC - 2)
            proj(NQC - 1)

        @block.scalar
        def _(scalar):
            # pure exp; pt is 4-deep so the pt anti-dependency (AV g-4 done)
            # is implied by waiting on scores(g), which the in-order tensor
            # queue issues after av(g-4)
            for g in range(NG):
                scalar.activation(pt_[g % 4][:, :], s_[g % 2][:, :], EXP)._wait_ge(
                    sSC, g + 1
                ).then_inc(sEX, 1)

        @block.vector
        def _(vector):
            vector.wait_ge(mset, NMSET)
            # production copies: PSUM fp32 -> SBUF bf16, q/k bias fused in
            for g in range(2 * NQC):
                c, which = g // 2, g % 2
                sl = slice(c * QC, (c + 1) * QC)
                if which == 0:  # q: split per head into the padded buffers
                    vector.tensor_scalar_add(
                        qTh0[0:Dh, sl], pq_[g % 2][0:Dh, :], tbq[0:Dh, 0:1]
                    )._wait_ge(sPQ, g + 1).then_inc(sCP, 1)
                    vector.tensor_scalar_add(
                        qTh1[Dh : 2 * Dh, sl],
                        pq_[g % 2][Dh : 2 * Dh, :],
                        tbq[Dh : 2 * Dh, 0:1],
                    ).then_inc(sCP, 1)
                else:
                    vector.tensor_scalar_add(
                        kT[0 : 2 * Dh, sl], pq_[g % 2][0 : 2 * Dh, :], tbk[:, 0:1]
                    )._wait_ge(sPQ, g + 1).then_inc(sCP, 1)
            for pb in range(NKB):
                o = pb * VW
                p = pq_[pb % 2]
                vector.tensor_copy(v_all[:, o : o + Dh], p[:, 0:Dh])._wait_ge(
                    sPQ, 2 * NQC + pb + 1
                ).then_inc(sCP, 1)
                vector.tensor_copy(
                    v_all[:, o + Dh + 1 : o + 2 * Dh + 1], p[:, Dh : 2 * Dh]
                ).then_inc(sCP, 1)
            # attention epilogue chain + projection copies
            for i in range(NI):
                c, h = i // 2, i % 2
                qsl = slice(c * QC, (c + 1) * QC)
                if i >= 3 and i % 2 == 1 and i < NI - 1:
                    cc = (i - 3) // 2
                    for mc in range(2):
                        k = 2 * cc + mc
                        if k >= 2:
                            vector.wait_ge((sOD0, sOD1)[k % 2], 16 * (k // 2))  # osb[k%2] free
                        vector.tensor_copy(osb_[k % 2][:, :], pq0[:, :])._wait_ge(
                            sPJ, k + 1
                        ).then_inc(sOS, 1)
                # den_r[i%4]'s prior reader bcast(i-4) precedes av(last of i)
                # on the in-order tensor queue, so waiting on sAV covers it
                vector.reciprocal(
                    den_r_[i % 4][0:1, :], po_[i % 2][Dh : Dh + 1, :]
                )._wait_ge(sAV, NT * (i + 1)).then_inc(sRC, 1)
                # den_sb[i%2]'s prior reader mul(i-2) is our own earlier
                # instruction (in-order DVE queue)
                vector.tensor_copy(den_sb_[i % 2][:, :], pq1[0:Dh, :])._wait_ge(
                    sBC, i + 1
                ).then_inc(sDC, 1)
                vector.tensor_mul(
                    oT_[h][0:Dh, qsl], po_[i % 2][0:Dh, :], den_sb_[i % 2][:, :]
                )._wait_ge(sDC, i + 1).then_inc(sMU, 1)
            for cc in (NQC - 2, NQC - 1):
                for mc in range(2):
                    k = 2 * cc + mc
                    vector.wait_ge((sOD0, sOD1)[k % 2], 16 * (k // 2))
                    vector.tensor_copy(osb_[k % 2][:, :], pq0[:, :])._wait_ge(
                        sPJ, k + 1
                    ).then_inc(sOS, 1)

    return nc


def _prep_in_maps(x, W_qkv, b_qkv, W_proj):
    in_maps = []
    for i in range(NCORES):
        b = i // 4
        heads = [2 * (i % 4), 2 * (i % 4) + 1]
        xT = np.ascontiguousarray(x[b].T.astype(np.float32))  # [256, 4096]

        def slc(base, scale):
            w = np.concatenate(
                [W_qkv[:, base + h * Dh : base + (h + 1) * Dh] for h in heads], axis=1
            ).astype(np.float32) * scale
            bb = np.concatenate(
                [b_qkv[base + h * Dh : base + (h + 1) * Dh] for h in heads]
            ).astype(np.float32) * scale
            return w, bb

        wq, bq = slc(0, SCALE)
        wk, bk = slc(C, 1.0)
        wv, bv = slc(2 * C, 1.0)
        wp = np.concatenate(
            [W_proj[h * Dh : (h + 1) * Dh, :] for h in heads], axis=0
        ).astype(np.float32)  # [64, 256]
        m = {
            "x0": np.ascontiguousarray(xT[:128]).astype(BF16),
            "x1": np.ascontiguousarray(xT[128:]).astype(BF16),
            "wq0": np.ascontiguousarray(wq[:128]).astype(BF16),
            "wq1": np.ascontiguousarray(wq[128:]).astype(BF16),
            "wqb": np.ascontiguousarray(bq[:, None]),
            "wk0": np.ascontiguousarray(wk[:128]).astype(BF16),
            "wk1": np.ascontiguousarray(wk[128:]).astype(BF16),
            "wkb": np.ascontiguousarray(bk[:, None]),
            "wv0": np.ascontiguousarray(wv[:128]).astype(BF16),
            "wv1": np.ascontiguousarray(wv[128:]).astype(BF16),
            "wvb": np.ascontiguousarray(bv[None, :]).astype(BF16),
            "wp0": np.ascontiguousarray(wp[:Dh]).astype(BF16),
            "wp1": np.ascontiguousarray(wp[Dh:]).astype(BF16),
        }
        in_maps.append(m)
    return in_maps


LAST_RESULT = None


def kernel(x, W_qkv, b_qkv, W_proj, b_proj):
    global LAST_RESULT
    from concourse.bass_utils import run_bass_kernel_spmd

    if "nc" not in _CACHE:
        _CACHE["nc"] = _build_nc()
    nc = _CACHE["nc"]

    in_maps = _prep_in_maps(
        np.asarray(x), np.asarray(W_qkv), np.asarray(b_qkv), np.asarray(W_proj)
    )
    res = run_bass_kernel_spmd(nc, in_maps, core_ids=list(range(NCORES)))
    LAST_RESULT = res
    outs = res.results
    full = np.zeros((B, N, C), dtype=np.float32)
    for i in range(NCORES):
        b = i // 4
        full[b] += np.asarray(outs[i]["out"]).T
    full += np.asarray(b_proj).astype(np.float32)[None, None, :]
    return full


# revision 44
# speedup vs baseline: 1.0532x; 1.0532x over previous
"""Multi-head attention kernel for 8 TRN2 NeuronCores (raw Bass, no Tile).

Problem: x[2,4096,256] -> qkv proj -> 8-head attention (Dh=32) -> out proj.
Sharding: 16 (batch, head) pairs over 8 cores: core i handles batch i//4 and
heads {2*(i%4), 2*(i%4)+1}. Each core computes its 2 heads' attention plus the
partial output projection; host sums the 4 partial projections per batch.

All matmul operands are bf16 (1 PE cycle/row vs 4 for fp32); PSUM accumulation
stays fp32. The softmax exp runs on ScalarE from PSUM (fp32 in, bf16 out) and
is the steady-state bottleneck (~1.1µs per [128,1024] tile).

Semaphore waits are EMBEDDED into the consuming instructions (one wait per
instruction; walrus moves matmul waits onto the paired LDWEIGHTS) instead of
standalone EVENT_SEMAPHORE instructions: gaps in the PE instruction stream
make the HAM activity monitor hold the PE clock at K=4/8 (1.2 GHz instead of
2.4), which doubled tensor time in the gap-heavy variant. pt (exp output) and
den_r are 4-deep so their anti-dependency waits are implied by the in-order
tensor queue and can be dropped entirely.

Per-core dataflow:
  xT [256,4096] bf16 (features on partitions); q/k biases added by DVE
  tensor_scalar_add during the PSUM->SBUF cast (no bias matmuls); softmax
  scale folded into Wq host-side.
  qT/kT [64,4096] bf16 (dh on partitions, head h at rows 32h:32h+32).
  v packed [128, 32*66] bf16: per k-block [v_h0(32)|1|v_h1(32)|1]: the ones
  column makes the AV matmul (M=33) also produce the softmax denominator row.
  Scores transposed: S^T[128 kpos, 2*512 q] = kT_blk.T @ qT_chunk (PSUM,
  double-buffered), exp on ScalarE -> pt bf16, AV: po[33,512] += v_blk.T @ P^T
  across 32 k-blocks, denominator reciprocal (reciprocal_approx_fast, ~18
  correct bits) read straight from PSUM, broadcast to 32 partitions via K=1
  ones matmul, normalize on VectorE (reads po PSUM directly), project, DMA.

Pipeline: PE runs scores(g+1) before AV(g) so ScalarE's exp(g) overlaps;
ping-pong pairs for score PSUM, po PSUM, and all epilogue buffers.
"""

import sys

sys.path.insert(0, "/opt/trn_rl_repo")

import numpy as np
import ml_dtypes

BF16 = ml_dtypes.bfloat16

B, N, C, H, Dh = 2, 4096, 256, 8, 32
HPC = 2  # heads per core
NCORES = 8
SCALE = C ** -0.5
QC = 512  # q columns per chunk
NQC = N // QC  # 8
KB = 128  # k rows per block
NKB = N // KB  # 32
VW = 2 * Dh + 2  # packed v width per k-block
NI = NQC * HPC  # 16 (c, h) iterations
NT = NKB // 2  # 16 double-kb tiles per iteration
NG = NI * NT  # 256 score/exp/AV tiles

_CACHE = {}


def _build_nc():
    import concourse.bass as bass
    import concourse.mybir as mybir

    f32 = mybir.dt.float32
    bf16 = mybir.dt.bfloat16
    EXP = mybir.ActivationFunctionType.Exp

    nc = bass.Bass("TRN2", target_bir_lowering=False, debug=False, num_devices=NCORES)

    ins = {}
    for nm, shp, dt in (
        ("x0", [128, N], bf16), ("x1", [128, N], bf16),
        ("wqkv", [128, 6 * 2 * Dh], bf16),  # q0|q1|k0|k1|v0|v1
        ("wb", [2 * Dh, 2], f32),  # q bias | k bias
        ("wvb", [1, 2 * Dh], bf16),
        ("wp", [Dh, 2 * C], bf16),  # wp0 | wp1
    ):
        ins[nm] = nc.dram_tensor(nm, shp, dt, kind="ExternalInput")
    out = nc.dram_tensor("out", [C, N], f32, kind="ExternalOutput")

    from contextlib import ExitStack
    with ExitStack() as ctx:
        E = ctx.enter_context
        E(nc.allow_low_precision(reason="bf16 matmul operands; fp32 PSUM accum"))
        xt0 = E(nc.sbuf_tensor("xt0", [128, N], bf16))
        xt1 = E(nc.sbuf_tensor("xt1", [128, N], bf16))
        tw_all = E(nc.sbuf_tensor("tw_all", [128, 6 * 2 * Dh], bf16))
        W = 2 * Dh
        twq0, twq1 = tw_all[:, 0:W], tw_all[:, W : 2 * W]
        twk0, twk1 = tw_all[:, 2 * W : 3 * W], tw_all[:, 3 * W : 4 * W]
        twv0, twv1 = tw_all[:, 4 * W : 5 * W], tw_all[:, 5 * W : 6 * W]
        tb = E(nc.sbuf_tensor("tb", [2 * Dh, 2], f32))
        tbq, tbk = tb[:, 0:1], tb[:, 1:2]
        twvb = E(nc.sbuf_tensor("twvb", [1, 2 * Dh], bf16))
        # proj weights / oT / bcast operands are also K=128 zero-padded (see
        # qTh comment: row-limited weights de-warm the HAM clock gate)
        twp = E(nc.sbuf_tensor("twp", [128, 2 * C], bf16))
        twp0, twp1 = twp[:, 0:C], twp[:, C : 2 * C]
        ones_sm = E(nc.sbuf_tensor("ones_sm", [1, QC], bf16))
        ones_bc = E(nc.sbuf_tensor("ones_bc", [128, Dh], bf16))
        # scores contraction is padded to K=128 with zero rows: matmuls whose
        # weights use only a 32-row strip of the PE array don't register as
        # "busy" to the HAM clock gate, which then holds the PE at K=4/8
        # (1.2 GHz). kT holds both heads' k at rows 0..63, zeros above; qTh[h]
        # holds head h's q at rows 32h..32h+32, zeros elsewhere, so the cross
        # terms vanish. Costs nothing: matmul time scales with moving columns.
        qTh0 = E(nc.sbuf_tensor("qTh0", [128, N], bf16))
        qTh1 = E(nc.sbuf_tensor("qTh1", [128, N], bf16))
        kT = E(nc.sbuf_tensor("kT", [128, N], bf16))
        qTh_ = (qTh0, qTh1)
        v_all = E(nc.sbuf_tensor("v_all", [128, NKB * VW], bf16))
        pt_ = tuple(
            E(nc.sbuf_tensor(f"pt{j}", [128, 2 * QC], bf16)) for j in range(4)
        )
        oT0 = E(nc.sbuf_tensor("oT0", [128, N], bf16))
        oT1 = E(nc.sbuf_tensor("oT1", [128, N], bf16))
        den_r_ = tuple(
            E(nc.sbuf_tensor(f"den_r{j}", [128, QC], bf16)) for j in range(4)
        )
        den_sb0 = E(nc.sbuf_tensor("den_sb0", [Dh, QC], f32))
        den_sb1 = E(nc.sbuf_tensor("den_sb1", [Dh, QC], f32))
        osb0 = E(nc.sbuf_tensor("osb0", [128, QC], f32))
        osb1 = E(nc.sbuf_tensor("osb1", [128, QC], f32))
        s0 = E(nc.psum_tensor("s0", [128, 2 * QC], f32))
        s1 = E(nc.psum_tensor("s1", [128, 2 * QC], f32))
        po0 = E(nc.psum_tensor("po0", [Dh + 1, QC], f32))
        po1 = E(nc.psum_tensor("po1", [Dh + 1, QC], f32))
        pq0 = E(nc.psum_tensor("pq0", [128, QC], f32))
        pq1 = E(nc.psum_tensor("pq1", [128, QC], f32))
        dIN = E(nc.semaphore("dIN"))
        mset = E(nc.semaphore("mset"))
        sPQ = E(nc.semaphore("sPQ"))
        sCP = E(nc.semaphore("sCP"))
        sSC = E(nc.semaphore("sSC"))
        sEX = E(nc.semaphore("sEX"))
        sAV = E(nc.semaphore("sAV"))
        sRC = E(nc.semaphore("sRC"))
        sBC = E(nc.semaphore("sBC"))
        sDC = E(nc.semaphore("sDC"))
        sMU = E(nc.semaphore("sMU"))
        sPJ = E(nc.semaphore("sPJ"))
        sOS = E(nc.semaphore("sOS"))
        sOD0 = E(nc.semaphore("sOD0"))
        sOD1 = E(nc.semaphore("sOD1"))
        block = E(nc.Block())

        s_ = (s0, s1)
        po_ = (po0, po1)
        pq_ = (pq0, pq1)
        # production-phase PSUM rotation: 4-deep using the (still unused)
        # score banks, so the PE never waits on the DVE drain ping-pong
        prod_ = (pq0[:, :], pq1[:, :], s0[:, 0:QC], s0[:, QC : 2 * QC])
        den_sb_ = (den_sb0, den_sb1)
        osb_ = (osb0, osb1)
        oT_ = (oT0, oT1)

        NMSET = 14

        @block.sync
        def _(sync):
            sync.dma_start(out=xt0[:, :], in_=ins["x0"].ap()).then_inc(dIN, 16)
            sync.dma_start(out=tw_all[:, :], in_=ins["wqkv"].ap()).then_inc(dIN, 16)
            sync.dma_start(out=tb[:, :], in_=ins["wb"].ap()).then_inc(dIN, 16)
            sync.dma_start(out=twvb[:, :], in_=ins["wvb"].ap()).then_inc(dIN, 16)
            for k in range(2 * NQC):  # out DMAs: c-major, mc-minor
                c, mc = k // 2, k % 2
                sync.wait_ge(sOS, k + 1)
                sync.dma_start(
                    out=out.ap()[mc * 128 : (mc + 1) * 128, c * QC : (c + 1) * QC],
                    in_=osb_[k % 2][:, :],
                ).then_inc((sOD0, sOD1)[k % 2], 16)

        @block.gpsimd
        def _(gpsimd):
            gpsimd.memset(ones_sm[:, :], 1.0).then_inc(mset, 1)
            gpsimd.memset(ones_bc[:, :], 0.0).then_inc(mset, 1)
            gpsimd.memset(ones_bc[0:1, :], 1.0).then_inc(mset, 1)
            gpsimd.memset(v_all[:, :], 1.0).then_inc(mset, 1)
            gpsimd.memset(qTh0[:, :], 0.0).then_inc(mset, 1)
            gpsimd.memset(qTh1[:, :], 0.0).then_inc(mset, 1)
            gpsimd.memset(kT[:, :], 0.0).then_inc(mset, 1)
            gpsimd.memset(oT0[:, :], 0.0).then_inc(mset, 1)
            gpsimd.memset(oT1[:, :], 0.0).then_inc(mset, 1)
            gpsimd.memset(twp[:, :], 0.0).then_inc(mset, 1)
            for j in range(4):
                gpsimd.memset(den_r_[j][:, :], 0.0).then_inc(mset, 1)
            # after the twp zero fill (same queue, in-order)
            gpsimd.dma_start(out=twp[0:Dh, :], in_=ins["wp"].ap()).then_inc(dIN, 16)

        # cumulative sCP count after production group g (q groups make 2
        # copies — one per head's padded buffer — k groups 1)
        cum = []
        tot = 0
        for g in range(2 * NQC):
            tot += 2 if g % 2 == 0 else 1
            cum.append(tot)
        CUM_QK = cum[-1]  # 24

        @block.tensor
        def _(tensor):
            tensor.wait_ge(dIN, 6 * 16)
            tensor.wait_ge(mset, NMSET)
            # ---- q/k projections: 16 groups (c-major; q then k) ----
            for g in range(2 * NQC):
                c, which = g // 2, g % 2
                sl = slice(c * QC, (c + 1) * QC)
                w0, w1 = ((twq0, twq1), (twk0, twk1))[which]
                p = prod_[g % 4][0 : 2 * Dh, :]
                mm = tensor.matmul(p, w0[:, :], xt0[:, sl], start=True, stop=False)
                if g >= 4:
                    mm._wait_ge(sCP, cum[g - 4])  # prod[g%4] free after copies g-4
                tensor.matmul(
                    p, w1[:, :], xt1[:, sl], start=False, stop=True
                ).then_inc(sPQ, 1)
            # ---- v projection: 32 blocks ----
            for pb in range(NKB):
                psl = slice(pb * KB, (pb + 1) * KB)
                p = prod_[pb % 4]
                pv = p[:, 0 : 2 * Dh]
                tensor.matmul(
                    pv, xt0[:, psl], twv0[:, :], start=True, stop=False
                )._wait_ge(sCP, CUM_QK + 2 * pb - 6 if pb >= 4 else CUM_QK)
                tensor.matmul(pv, xt1[:, psl], twv1[:, :], start=False, stop=False)
                tensor.matmul(
                    pv, ones_sm[0:1, 0:KB], twvb[:, :], start=False, stop=True
                ).then_inc(sPQ, 1)
            tensor.wait_ge(sCP, CUM_QK + 2 * NKB)  # all production copies done
            # ---- attention ----
            def scores(g):
                # no sEX wait: av(g-2), issued just before us in program
                # order, already waited on sEX >= g-1 (s[g%2] free)
                i, t = g // NT, g % NT
                c, h = i // 2, i % 2
                qsl = slice(c * QC, (c + 1) * QC)
                for j in (0, 1):
                    kb = 2 * t + j
                    ksl = slice(kb * KB, (kb + 1) * KB)
                    mm = tensor.matmul(
                        s_[g % 2][:, j * QC : (j + 1) * QC],
                        kT[:, ksl],
                        qTh_[h][:, qsl],
                        start=True,
                        stop=True,
                    )
                    if j == 1:
                        mm.then_inc(sSC, 1)

            def av(g):
                i, t = g // NT, g % NT
                h = i % 2
                if t == 0 and i >= 2:
                    tensor.wait_ge(sMU, i - 1)  # po[i%2] free after mul i-2
                for j in (0, 1):
                    kb = 2 * t + j
                    o = kb * VW + (Dh + 1) * h
                    mm = tensor.matmul(
                        po_[i % 2][:, :],
                        v_all[:, o : o + Dh + 1],
                        pt_[g % 4][:, j * QC : (j + 1) * QC],
                        start=(kb == 0),
                        stop=(kb == NKB - 1),
                        skip_group_check=True,
                    )
                    if j == 0:
                        mm._wait_ge(sEX, g + 1)  # exp g done
                    if j == 1:
                        mm.then_inc(sAV, 1)

            def bcast(i):
                if i >= 1:
                    tensor.wait_ge(sDC, i)  # pq1 free after den copy i-1
                tensor.matmul(
                    pq1[0:Dh, :],
                    ones_bc[:, :],
                    den_r_[i % 4][:, :],
                    start=True,
                    stop=True,
                )._wait_ge(sRC, i + 1).then_inc(sBC, 1)

            def proj_mc(c, mc):
                qsl = slice(c * QC, (c + 1) * QC)
                if mc == 0:
                    tensor.wait_ge(sMU, 2 * c + 2)  # oT ready through chunk c
                k = 2 * c + mc
                msl = slice(mc * 128, (mc + 1) * 128)
                mm = tensor.matmul(
                    pq0[:, :], twp0[:, msl], oT0[:, qsl], start=True, stop=False
                )
                if k >= 1:
                    mm._wait_ge(sOS, k)  # pq0 free after osb copy k-1
                tensor.matmul(
                    pq0[:, :], twp1[:, msl], oT1[:, qsl], start=False, stop=True
                ).then_inc(sPJ, 1)

            def proj(c):
                proj_mc(c, 0)
                proj_mc(c, 1)

            for i in range(NI):
                for t in range(NT):
                    g = i * NT + t
                    scores(g)
                    if t >= 1:
                        av(g - 1)
                    if t == 4 and i >= 1:
                        # t==4: late enough that the ~3.3us DVE reciprocal of
                        # iter i-1 has finished (no PE stall on sRC), early
                        # enough that mul(i-1) lands this iteration, one full
                        # iteration before av(i+1, t=0) waits on sMU
                        bcast(i - 1)
                    # spread the proj matmuls so ScalarE's exp pipeline is
                    # not starved by a bunched block of PE work at the
                    # iteration boundary
                    if i >= 3 and i % 2 == 1 and i < NI - 1:
                        if t == 8:
                            proj_mc((i - 3) // 2, 0)
                        elif t == 12:
                            proj_mc((i - 3) // 2, 1)
                av(i * NT + NT - 1)
            bcast(NI - 1)
            proj(NQC - 2)
            proj(NQC - 1)

        @block.scalar
        def _(scalar):
            scalar.dma_start(out=xt1[:, :], in_=ins["x1"].ap()).then_inc(dIN, 16)
            # pure exp; pt is 4-deep so the pt anti-dependency (AV g-4 done)
            # is implied by waiting on scores(g), which the in-order tensor
            # queue issues after av(g-4)
            for g in range(NG):
                scalar.activation(pt_[g % 4][:, :], s_[g % 2][:, :], EXP)._wait_ge(
                    sSC, g + 1
                ).then_inc(sEX, 1)

        @block.vector
        def _(vector):
            vector.wait_ge(mset, NMSET)
            # production copies: PSUM fp32 -> SBUF bf16, q/k bias fused in
            for g in range(2 * NQC):
                c, which = g // 2, g % 2
                sl = slice(c * QC, (c + 1) * QC)
                if which == 0:  # q: split per head into the padded buffers
                    vector.tensor_scalar_add(
                        qTh0[0:Dh, sl], prod_[g % 4][0:Dh, :], tbq[0:Dh, 0:1]
                    )._wait_ge(sPQ, g + 1).then_inc(sCP, 1)
                    vector.tensor_scalar_add(
                        qTh1[Dh : 2 * Dh, sl],
                        prod_[g % 4][Dh : 2 * Dh, :],
                        tbq[Dh : 2 * Dh, 0:1],
                    ).then_inc(sCP, 1)
                else:
                    vector.tensor_scalar_add(
                        kT[0 : 2 * Dh, sl], prod_[g % 4][0 : 2 * Dh, :], tbk[:, 0:1]
                    )._wait_ge(sPQ, g + 1).then_inc(sCP, 1)
            for pb in range(NKB):
                o = pb * VW
                p = prod_[pb % 4]
                vector.tensor_copy(v_all[:, o : o + Dh], p[:, 0:Dh])._wait_ge(
                    sPQ, 2 * NQC + pb + 1
                ).then_inc(sCP, 1)
                vector.tensor_copy(
                    v_all[:, o + Dh + 1 : o + 2 * Dh + 1], p[:, Dh : 2 * Dh]
                ).then_inc(sCP, 1)
            # attention epilogue chain + projection copies
            for i in range(NI):
                c, h = i // 2, i % 2
                qsl = slice(c * QC, (c + 1) * QC)
                if i >= 3 and i % 2 == 1 and i < NI - 1:
                    cc = (i - 3) // 2
                    for mc in range(2):
                        k = 2 * cc + mc
                        if k >= 2:
                            vector.wait_ge((sOD0, sOD1)[k % 2], 16 * (k // 2))  # osb[k%2] free
                        vector.tensor_copy(osb_[k % 2][:, :], pq0[:, :])._wait_ge(
                            sPJ, k + 1
                        ).then_inc(sOS, 1)
                # den_r[i%4]'s prior reader bcast(i-4) precedes av(last of i)
                # on the in-order tensor queue, so waiting on sAV covers it
                vector.reciprocal(
                    den_r_[i % 4][0:1, :], po_[i % 2][Dh : Dh + 1, :]
                )._wait_ge(sAV, NT * (i + 1)).then_inc(sRC, 1)
                # den_sb[i%2]'s prior reader mul(i-2) is our own earlier
                # instruction (in-order DVE queue)
                vector.tensor_copy(den_sb_[i % 2][:, :], pq1[0:Dh, :])._wait_ge(
                    sBC, i + 1
                ).then_inc(sDC, 1)
                vector.tensor_mul(
                    oT_[h][0:Dh, qsl], po_[i % 2][0:Dh, :], den_sb_[i % 2][:, :]
                )._wait_ge(sDC, i + 1).then_inc(sMU, 1)
            for cc in (NQC - 2, NQC - 1):
                for mc in range(2):
                    k = 2 * cc + mc
                    vector.wait_ge((sOD0, sOD1)[k % 2], 16 * (k // 2))
                    vector.tensor_copy(osb_[k % 2][:, :], pq0[:, :])._wait_ge(
                        sPJ, k + 1
                    ).then_inc(sOS, 1)

    return nc


def _prep_in_maps(x, W_qkv, b_qkv, W_proj):
    in_maps = []
    for i in range(NCORES):
        b = i // 4
        heads = [2 * (i % 4), 2 * (i % 4) + 1]
        xT = np.ascontiguousarray(x[b].T.astype(np.float32))  # [256, 4096]

        def slc(base, scale):
            w = np.concatenate(
                [W_qkv[:, base + h * Dh : base + (h + 1) * Dh] for h in heads], axis=1
            ).astype(np.float32) * scale
            bb = np.concatenate(
                [b_qkv[base + h * Dh : base + (h + 1) * Dh] for h in heads]
            ).astype(np.float32) * scale
            return w, bb

        wq, bq = slc(0, SCALE)
        wk, bk = slc(C, 1.0)
        wv, bv = slc(2 * C, 1.0)
        wp = np.concatenate(
            [W_proj[h * Dh : (h + 1) * Dh, :] for h in heads], axis=0
        ).astype(np.float32)  # [64, 256]
        wqkv = np.hstack([wq[:128], wq[128:], wk[:128], wk[128:], wv[:128], wv[128:]])
        m = {
            "x0": np.ascontiguousarray(xT[:128]).astype(BF16),
            "x1": np.ascontiguousarray(xT[128:]).astype(BF16),
            "wqkv": np.ascontiguousarray(wqkv).astype(BF16),
            "wb": np.ascontiguousarray(np.stack([bq, bk], axis=1)),
            "wvb": np.ascontiguousarray(bv[None, :]).astype(BF16),
            "wp": np.ascontiguousarray(np.hstack([wp[:Dh], wp[Dh:]])).astype(BF16),
        }
        in_maps.append(m)
    return in_maps


LAST_RESULT = None


def kernel(x, W_qkv, b_qkv, W_proj, b_proj):
    global LAST_RESULT
    from concourse.bass_utils import run_bass_kernel_spmd

    if "nc" not in _CACHE:
        _CACHE["nc"] = _build_nc()
    nc = _CACHE["nc"]

    in_maps = _prep_in_maps(
        np.asarray(x), np.asarray(W_qkv), np.asarray(b_qkv), np.asarray(W_proj)
    )
    res = run_bass_kernel_spmd(nc, in_maps, core_ids=list(range(NCORES)))
    LAST_RESULT = res
    outs = res.results
    full = np.zeros((B, N, C), dtype=np.float32)
    for i in range(NCORES):
        b = i // 4
        full[b] += np.asarray(outs[i]["out"]).T
    full += np.asarray(b_proj).astype(np.float32)[None, None, :]
    return full
